# revision 25
# baseline (speedup 1.0000x reference)
"""Trainium2 Bass kernel for batched windowed DFT (STFT-as-GEMM).

Problem: for each batch row of x (8, 262144), reflect-pad by 1024, frame into
513 overlapping windows (len 2048, hop 512), and multiply by dense Hann-windowed
sin/cos DFT matrices (2048x2048):  real = wcos @ frames^T, out = (real, -imag).

Strategy (one batch per NeuronCore, 8 cores):
  * Hermitian symmetry of the real-input DFT: compute bins k=0..1151 only;
    rows 1152..2047 are mirrors (real: copy, imag: sign flip) of rows 896..1.
    The imag mirror is derived on host from the shipped rows (no extra DMA).
  * Even/odd fold of the contraction: the Hann-windowed basis obeys
    w[k, 2048-n] = +/- w[k, n], so contract only n=1..1023 against folded
    frames S+- = f[n] -+ f[2048-n] (computed on-chip by the vector engine).
    The n=1024 edge term is a rank-1 tile (+-1)^k * f[1024+512t] computed once
    on the PE and added during PSUM evacuation.
  * fp16 everywhere off-chip (signal in, weights, spectra out) -- halves DMA;
    fp32 PSUM accumulation on-chip keeps error ~5e-4.
  * 257/256 column split per PSUM bank pair (no 2-wide tail matmuls).
  * Outputs stream out in 3 kt-waves overlapped with later tiles' compute.
"""
import numpy as np

import concourse.bacc as bacc
import concourse.mybir as mybir
import concourse.tile as tile

F32 = mybir.dt.float32
F16 = mybir.dt.float16
T = 513          # frames
TP = 514         # padded column pitch for the folded-signal tiles
PH = 520         # per-phase column pitch of the phase-split signal layout
NKT = 9          # k tiles computed directly: k = 0..1151
NA = 8           # folded contraction chunks of 128 (n = 0..1023)
N_CORES = 8
L_PAD = 264192   # 262144 + 2*1024
C0, C1 = 257, 256  # frame split across the two PSUM banks of a group


def build_nc(reps=1):
    nc = bacc.Bacc("TRN2", target_bir_lowering=False, debug=False,
                   num_devices=N_CORES)
    xa_d = nc.dram_tensor("xa", [128, 4 * PH], F16, kind="ExternalInput")
    xr_d = nc.dram_tensor("xr", [128, 4 * PH], F16, kind="ExternalInput")
    wt_d = nc.dram_tensor("wt", [NKT * 128, 2048], F16, kind="ExternalInput")
    sgn_d = nc.dram_tensor("sgn", [128, 128], F16, kind="ExternalInput")
    # p-major spectra: row p*2*NKT + s*NKT + kt holds bins (s, kt*128+p);
    # p-major keeps each output wave's DMA runs contiguous (nw*1026B per
    # partition instead of 1026B), host unpermutes for free.
    out_d = nc.dram_tensor("outRI", [128 * 2 * NKT, T], F16,
                           kind="ExternalOutput")

    with tile.TileContext(nc) as tc:
        with (
            tc.tile_pool(name="xin", bufs=1) as xin,
            tc.tile_pool(name="wts", bufs=1) as wts,
            tc.tile_pool(name="sbf", bufs=1) as sbf,
            tc.tile_pool(name="stag", bufs=1) as stag,
            tc.tile_pool(name="ps", bufs=3, space="PSUM") as ps,
            tc.tile_pool(name="pse", bufs=1, space="PSUM") as pse,
        ):
            XA = xin.tile([128, 4 * PH], F16, tag="XA")
            XR = xin.tile([128, 4 * PH], F16, tag="XR")
            SG = xin.tile([128, 128], F16, tag="SG")
            W = wts.tile([128, NKT * 2048], F16, tag="W")
            Sm = sbf.tile([128, NA * TP], F16, tag="Sm")
            Sp = sbf.tile([128, NA * TP], F16, tag="Sp")
            Ev = sbf.tile([128, T], F16, tag="Ev")
            stCN = stag.tile([128, 2 * NKT * T], F16, tag="stCN")

            for _rep in range(reps):
                # One HWDGE ring, explicit order: W block 0 first (first-group
                # matmuls need it), then signal halves in fold order (folds
                # a in {0,1,4,5} need only XA phases 0-1 + XR phases 2-3),
                # then the remaining weight blocks.
                nc.sync.dma_start(W[:, 0:2048], wt_d.ap()[0:128, :])
                nc.sync.dma_start(XA[:, 0:PH], xa_d.ap()[:, 0:PH])
                nc.sync.dma_start(XR[:, 3 * PH:4 * PH],
                                  xr_d.ap()[:, 3 * PH:4 * PH])
                nc.sync.dma_start(SG[:], sgn_d.ap())
                nc.sync.dma_start(XA[:, PH:2 * PH], xa_d.ap()[:, PH:2 * PH])
                nc.sync.dma_start(XR[:, 2 * PH:3 * PH],
                                  xr_d.ap()[:, 2 * PH:3 * PH])
                nc.sync.dma_start(XA[:, 2 * PH:4 * PH],
                                  xa_d.ap()[:, 2 * PH:4 * PH])
                nc.sync.dma_start(XR[:, 0:2 * PH], xr_d.ap()[:, 0:2 * PH])
                for kt in range(1, NKT):
                    nc.sync.dma_start(W[:, kt * 2048:(kt + 1) * 2048],
                                      wt_d.ap()[kt * 128:(kt + 1) * 128, :])

                # folds: S-+[a][p, t] = f[128a+p] -+ f[2048-(128a+p)] per frame t
                AORD = [0, 4, 1, 5, 2, 3, 6, 7]  # DMA-arrival order
                for a in AORD:
                    xa_s = XA[:, (a % 4) * PH + a // 4:(a % 4) * PH + a // 4 + TP]
                    ph = (15 - a) % 4
                    off = (15 - a) // 4
                    xr_s = XR[:, ph * PH + off:ph * PH + off + TP]
                    nc.vector.tensor_sub(Sm[:, a * TP:a * TP + TP], xa_s, xr_s)
                    nc.vector.tensor_add(Sp[:, a * TP:a * TP + TP], xa_s, xr_s)

                # rank-1 edge tile Ev[p, t] = (-1)^p * xp[512t + 1024], built
                # once on the PE (SG row 0 = (-1)^p) and added to every cos
                # group's evacuation: n=1024 term, wcos[k,1024] = (-1)^k.
                eps = pse.tile([128, 1024], F32, tag="eps")
                nc.tensor.matmul(eps[:, 0:C0], SG[:], XA[:, 2:2 + C0],
                                 start=True, stop=True)
                nc.tensor.matmul(eps[:, 512:512 + C1], SG[:],
                                 XA[:, 2 + C0:2 + T], start=True, stop=True)
                nc.vector.tensor_copy(Ev[:, 0:C0], eps[:, 0:C0])
                nc.vector.tensor_copy(Ev[:, C0:T], eps[:, 512:512 + C1])

                for kt in range(NKT):
                    for ch in range(2):  # 0 = sin (imag), 1 = cos (real)
                        acc = ps.tile([128, 1024], F32, tag="acc")
                        S = Sp if ch else Sm
                        col = kt * T if ch == 1 else NKT * T + kt * T
                        last = kt == NKT - 1
                        if last and ch == 1:
                            # bank-split order: finish PSUM bank A first so its
                            # evacuation + DMA overlap bank B's matmuls -- only
                            # a 65KB DMA then trails the final matmul
                            for i, a in enumerate(AORD):
                                wo = kt * 2048 + a * 256 + ch * 128
                                nc.tensor.matmul(acc[:, 0:C0],
                                                 W[:, wo:wo + 128],
                                                 S[:, a * TP:a * TP + C0],
                                                 start=(i == 0),
                                                 stop=(i == NA - 1))
                            nc.vector.tensor_add(stCN[:, col:col + C0],
                                                 acc[:, 0:C0], Ev[:, 0:C0])
                            dst = out_d.ap().rearrange(
                                "(p sk) t -> p sk t", sk=2 * NKT)[:, kt:kt + 1, :]
                            nc.sync.dma_start(
                                dst[:, :, 0:C0],
                                stCN[:, col:col + C0].rearrange(
                                    "p (kt t) -> p kt t", kt=1))
                            for i, a in enumerate(AORD):
                                wo = kt * 2048 + a * 256 + ch * 128
                                nc.tensor.matmul(acc[:, 512:512 + C1],
                                                 W[:, wo:wo + 128],
                                                 S[:, a * TP + C0:a * TP + T],
                                                 start=(i == 0),
                                                 stop=(i == NA - 1))
                            nc.vector.tensor_add(stCN[:, col + C0:col + T],
                                                 acc[:, 512:512 + C1],
                                                 Ev[:, C0:T])
                            nc.sync.dma_start(
                                dst[:, :, C0:T],
                                stCN[:, col + C0:col + T].rearrange(
                                    "p (kt t) -> p kt t", kt=1))
                            continue
                        for i, a in enumerate(AORD):
                            wo = kt * 2048 + a * 256 + ch * 128
                            lhsT = W[:, wo:wo + 128]
                            nc.tensor.matmul(acc[:, 0:C0], lhsT,
                                             S[:, a * TP:a * TP + C0],
                                             start=(i == 0), stop=(i == NA - 1))
                            nc.tensor.matmul(acc[:, 512:512 + C1], lhsT,
                                             S[:, a * TP + C0:a * TP + T],
                                             start=(i == 0), stop=(i == NA - 1))
                        if ch == 1:
                            nc.vector.tensor_add(stCN[:, col:col + C0],
                                                 acc[:, 0:C0], Ev[:, 0:C0])
                            nc.vector.tensor_add(stCN[:, col + C0:col + T],
                                                 acc[:, 512:512 + C1],
                                                 Ev[:, C0:T])
                        else:
                            nc.scalar.mul(stCN[:, col:col + C0],
                                          acc[:, 0:C0], -1.0)
                            nc.scalar.mul(stCN[:, col + C0:col + T],
                                          acc[:, 512:512 + C1], -1.0)
                            if last:  # sin section ships during cos matmuls
                                dst = out_d.ap().rearrange(
                                    "(p sk) t -> p sk t", sk=2 * NKT)[
                                    :, NKT + kt:NKT + kt + 1, :]
                                nc.sync.dma_start(
                                    dst, stCN[:, col:col + T].rearrange(
                                        "p (kt t) -> p kt t", kt=1))
                    # stream finished k-tiles out (one DMA covers the cos and
                    # sin sections); waves shrink near the end so the last DMA
                    # tail after the final matmul is small
                    waves = {2: (0, 3), 5: (3, 3), 7: (6, 2)}
                    if kt in waves:
                        w0, nw = waves[kt]
                        for s in range(2):
                            cols = slice(s * NKT * T + w0 * T,
                                         s * NKT * T + (w0 + nw) * T)
                            dst = out_d.ap().rearrange(
                                "(p sk) t -> p sk t", sk=2 * NKT)[
                                :, s * NKT + w0:s * NKT + w0 + nw, :]
                            nc.sync.dma_start(
                                dst, stCN[:, cols].rearrange(
                                    "p (kt t) -> p kt t", kt=nw))
    nc.compile()
    return nc


def host_prep(x, wsin, wcos):
    """Marshal full inputs into per-core input maps (pure data movement)."""
    x = np.asarray(x, dtype=np.float32)
    B = x.shape[0]
    xp = np.pad(x, ((0, 0), (1024, 1024)), mode="reflect")
    # layout A: XA[p, j] = xp[128j + p]
    XA = np.ascontiguousarray(xp.reshape(B, 2064, 128).transpose(0, 2, 1))
    # reversed layout: XR[p, j] = xp[128(j+1) - p] (out-of-range -> 0, unused)
    j = np.arange(2068); p = np.arange(128)
    idx = 128 * (j[None, :] + 1) - p[:, None]
    oob = idx >= L_PAD
    idx = np.where(oob, 0, idx)
    XR = xp[:, idx]
    XR[:, oob] = 0.0

    def phase_split(M, pitch=PH):
        B_, P_, C = M.shape
        out = np.zeros((B_, P_, 4, pitch), dtype=np.float32)
        for ph in range(4):
            col = M[:, :, ph::4]
            out[:, :, ph, :col.shape[2]] = col
        return np.ascontiguousarray(
            out.reshape(B_, P_, 4 * pitch).astype(np.float16))

    XA4 = phase_split(XA)
    XR4 = phase_split(XR)
    WT = np.concatenate([np.asarray(wsin).T[:1024, :NKT * 128],
                         np.asarray(wcos).T[:1024, :NKT * 128]],
                        axis=1).astype(np.float16)          # (1024, 2304)
    # block by k-tile so each 512KB block is one DMA and group kt only
    # depends on its own block: WH[kt*128+p, a*256+ch*128+c]
    WH = WT.reshape(NA, 128, 2, NKT, 128).transpose(3, 1, 0, 2, 4)
    WT = np.ascontiguousarray(WH.reshape(NKT * 128, 2048))
    sgn = np.zeros((128, 128), dtype=np.float16)
    sgn[0, :] = (-1.0) ** np.arange(128)
    return [{"xa": XA4[b], "xr": XR4[b], "wt": WT, "sgn": sgn} for b in range(B)]


def assemble(results):
    """Gather per-core outputs into the full (real, -imag) pair."""
    B = len(results)
    R = np.empty((B, 2048, T), np.float32)
    I = np.empty((B, 2048, T), np.float32)
    for b in range(B):
        # p-major rows: row p*2*NKT + s*NKT + kt -> bin (s, kt*128 + p)
        o = results[b]["outRI"].astype(np.float32)
        o = o.reshape(128, 2, NKT, T).transpose(1, 2, 0, 3).reshape(2, 1152, T)
        oR, oI = o[0], o[1]
        R[b, :1152] = oR
        R[b, 1152:] = oR[896:0:-1]       # cos mirror: copy
        I[b, :1152] = oI
        I[b, 1152:] = -oI[896:0:-1]      # sin mirror: negate of -imag rows
    return R, I


class _Runner:
    """Build once, jit once, run many (shard_map over the 8 cores)."""

    def __init__(self, reps=1):
        import jax
        from jax.sharding import Mesh, PartitionSpec
        from jax.experimental.shard_map import shard_map
        from concourse.bass2jax import _bass_exec_p, install_neuronx_cc_hook

        install_neuronx_cc_hook()
        self.jax = jax
        nc = build_nc(reps=reps)
        self.nc = nc
        in_names, out_names, out_avals = [], [], []
        for alloc in nc.m.functions[0].allocations:
            if not isinstance(alloc, mybir.MemoryLocationSet):
                continue
            name = alloc.memorylocations[0].name
            if alloc.kind == "ExternalInput":
                in_names.append(name)
            elif alloc.kind == "ExternalOutput":
                out_names.append(name)
                out_avals.append(jax.core.ShapedArray(
                    tuple(alloc.tensor_shape), mybir.dt.np(alloc.dtype)))
        self.in_names, self.out_names, self.out_avals = in_names, out_names, out_avals
        n_params = len(in_names)
        all_names = in_names + out_names

        def _body(*args):
            outs = _bass_exec_p.bind(
                *args,
                out_avals=tuple(out_avals),
                in_names=tuple(all_names),
                out_names=tuple(out_names),
                lowering_input_output_aliases=(),
                sim_require_finite=True,
                sim_require_nnan=True,
                nc=nc,
            )
            return tuple(outs)

        devices = jax.devices()[:N_CORES]
        mesh = Mesh(np.asarray(devices), ("core",))
        n_outs = len(out_names)
        self._fn = jax.jit(
            shard_map(_body, mesh=mesh,
                      in_specs=(PartitionSpec("core"),) * (n_params + n_outs),
                      out_specs=(PartitionSpec("core"),) * n_outs,
                      check_rep=False),
            keep_unused=True,
        )
        self._zeros = [np.zeros((N_CORES * a.shape[0], *a.shape[1:]), a.dtype)
                       for a in out_avals]

    def prepare(self, in_maps):
        pid = self.nc.partition_id_tensor.name if self.nc.partition_id_tensor else None
        in_maps = [
            dict(m, **({pid: np.array([[c]], dtype=np.uint32)} if pid else {}))
            for c, m in enumerate(in_maps)
        ]
        concat = [np.concatenate([np.asarray(m[name]) for m in in_maps], axis=0)
                  for name in self.in_names]
        self._args = [self.jax.device_put(a) for a in concat + self._zeros]
        self.jax.block_until_ready(self._args)

    def run(self):
        out = self._fn(*self._args)
        self.jax.block_until_ready(out)
        return out

    def results(self, out):
        res = []
        for c in range(N_CORES):
            d = {}
            for i, name in enumerate(self.out_names):
                a = np.asarray(out[i])
                d[name] = a.reshape(N_CORES, *self.out_avals[i].shape)[c]
            res.append(d)
        return res


_RUNNER = None


def kernel(x, wsin, wcos):
    """Full inputs in, full output out: returns (real, -imag) as in reference."""
    global _RUNNER
    if _RUNNER is None:
        _RUNNER = _Runner(reps=1)
    ins = host_prep(x, wsin, wcos)
    _RUNNER.prepare(ins)
    out = _RUNNER.run()
    R, I = assemble(_RUNNER.results(out))
    return R, I


# revision 35
# speedup vs baseline: 1.0565x; 1.0565x over previous
"""Trainium2 Bass kernel for batched windowed DFT (STFT-as-GEMM).

Problem: for each batch row of x (8, 262144), reflect-pad by 1024, frame into
513 overlapping windows (len 2048, hop 512), and multiply by dense Hann-windowed
sin/cos DFT matrices (2048x2048):  real = wcos @ frames^T, out = (real, -imag).

Strategy (one batch per NeuronCore, 8 cores):
  * Hermitian symmetry of the real-input DFT: compute bins k=0..1151 only;
    rows 1152..2047 are mirrors (real: copy, imag: sign flip) of rows 896..1.
    The imag mirror is derived on host from the shipped rows (no extra DMA).
  * Even/odd fold of the contraction: the Hann-windowed basis obeys
    w[k, 2048-n] = +/- w[k, n], so contract only n=1..1023 against folded
    frames S+- = f[n] -+ f[2048-n] (computed on-chip by the vector engine).
    The n=1024 edge term is a rank-1 tile (+-1)^k * f[1024+512t] computed once
    on the PE and added during PSUM evacuation.
  * fp16 everywhere off-chip (signal in, weights, spectra out) -- halves DMA;
    fp32 PSUM accumulation on-chip keeps error ~5e-4.
  * 257/256 column split per PSUM bank pair (no 2-wide tail matmuls).
  * Outputs stream out in 3 kt-waves overlapped with later tiles' compute.
"""
import numpy as np

import concourse.bacc as bacc
import concourse.mybir as mybir
import concourse.tile as tile

F32 = mybir.dt.float32
F16 = mybir.dt.float16
T = 513          # frames
TP = 514         # padded column pitch for the folded-signal tiles
PH = 520         # per-phase column pitch of the phase-split signal layout
NKT = 9          # k tiles computed directly: k = 0..1151
NA = 8           # folded contraction chunks of 128 (n = 0..1023)
N_CORES = 8
L_PAD = 264192   # 262144 + 2*1024
C0, C1 = 257, 256  # frame split across the two PSUM banks of a group


def build_nc(reps=1):
    nc = bacc.Bacc("TRN2", target_bir_lowering=False, debug=False,
                   num_devices=N_CORES)
    xa_d = nc.dram_tensor("xa", [128, 4 * PH], F16, kind="ExternalInput")
    xr_d = nc.dram_tensor("xr", [128, 4 * PH], F16, kind="ExternalInput")
    wt_d = nc.dram_tensor("wt", [8 * 128, 2048], F16, kind="ExternalInput")
    sgn_d = nc.dram_tensor("sgn", [128, 128], F16, kind="ExternalInput")
    # p-major spectra: row p*2*NKT + s*NKT + kt holds bins (s, kt*128+p);
    # p-major keeps each output wave's DMA runs contiguous (nw*1026B per
    # partition instead of 1026B), host unpermutes for free.
    out_d = nc.dram_tensor("outRI", [128 * 2 * NKT, T], F16,
                           kind="ExternalOutput")

    with tile.TileContext(nc) as tc:
        with (
            tc.tile_pool(name="xin", bufs=1) as xin,
            tc.tile_pool(name="wts", bufs=1) as wts,
            tc.tile_pool(name="sbf", bufs=1) as sbf,
            tc.tile_pool(name="stag", bufs=1) as stag,
            tc.tile_pool(name="ps", bufs=3, space="PSUM") as ps,
            tc.tile_pool(name="pse", bufs=1, space="PSUM") as pse,
        ):
            XA = xin.tile([128, 4 * PH], F16, tag="XA")
            XR = xin.tile([128, 4 * PH], F16, tag="XR")
            SG = xin.tile([128, 128], F16, tag="SG")
            W = wts.tile([128, 8 * 2048], F16, tag="W")
            Sm = sbf.tile([128, NA * TP], F16, tag="Sm")
            Sp = sbf.tile([128, NA * TP], F16, tag="Sp")
            Ev = sbf.tile([128, T], F16, tag="Ev")
            Od = sbf.tile([128, T], F32, tag="Od")  # accO staging (kt0 split)
            stCN = stag.tile([128, 2 * NKT * T], F16, tag="stCN")

            for _rep in range(reps):
                # One HWDGE ring, explicit order: W block 0 first (first-group
                # matmuls need it), then signal halves in fold order (folds
                # a in {0,1,4,5} need only XA phases 0-1 + XR phases 2-3),
                # then the remaining weight blocks.
                nc.sync.dma_start(W[:, 0:2048], wt_d.ap()[0:128, :])
                nc.sync.dma_start(XA[:, 0:PH], xa_d.ap()[:, 0:PH])
                nc.sync.dma_start(XR[:, 3 * PH:4 * PH],
                                  xr_d.ap()[:, 3 * PH:4 * PH])
                nc.sync.dma_start(SG[:], sgn_d.ap())
                nc.sync.dma_start(XA[:, PH:2 * PH], xa_d.ap()[:, PH:2 * PH])
                nc.sync.dma_start(XR[:, 2 * PH:3 * PH],
                                  xr_d.ap()[:, 2 * PH:3 * PH])
                nc.sync.dma_start(XA[:, 2 * PH:4 * PH],
                                  xa_d.ap()[:, 2 * PH:4 * PH])
                nc.sync.dma_start(XR[:, 0:2 * PH], xr_d.ap()[:, 0:2 * PH])
                for kt in range(1, 8):
                    nc.sync.dma_start(W[:, kt * 2048:(kt + 1) * 2048],
                                      wt_d.ap()[kt * 128:(kt + 1) * 128, :])

                # folds: S-+[a][p, t] = f[128a+p] -+ f[2048-(128a+p)] per frame t
                AORD = [0, 4, 1, 5, 2, 3, 6, 7]  # DMA-arrival order
                for a in AORD:
                    xa_s = XA[:, (a % 4) * PH + a // 4:(a % 4) * PH + a // 4 + TP]
                    ph = (15 - a) % 4
                    off = (15 - a) // 4
                    xr_s = XR[:, ph * PH + off:ph * PH + off + TP]
                    nc.vector.tensor_sub(Sm[:, a * TP:a * TP + TP], xa_s, xr_s)
                    nc.vector.tensor_add(Sp[:, a * TP:a * TP + TP], xa_s, xr_s)

                # rank-1 edge tile Ev[p, t] = (-1)^p * xp[512t + 1024], built
                # once on the PE (SG row 0 = (-1)^p) and added to every cos
                # group's evacuation: n=1024 term, wcos[k,1024] = (-1)^k.
                eps = pse.tile([128, 1024], F32, tag="eps")
                nc.tensor.matmul(eps[:, 0:C0], SG[:], XA[:, 2:2 + C0],
                                 start=True, stop=True)
                nc.tensor.matmul(eps[:, 512:512 + C1], SG[:],
                                 XA[:, 2 + C0:2 + T], start=True, stop=True)
                nc.vector.tensor_copy(Ev[:, 0:C0], eps[:, 0:C0])
                nc.vector.tensor_copy(Ev[:, C0:T], eps[:, 512:512 + C1])

                # ---- k-tile 0 with even/odd parity split: the contraction
                # rows are host-permuted so partitions 0..63 carry even n and
                # 64..127 odd n; K=64 matmuls to PE row-groups {0,1} / {2,3}
                # run concurrently, and k-tile 8 = E - O comes out free
                # (w[k+1024, n] = (-1)^n w[k, n]).
                for ch in range(2):  # 0 = sin (imag), 1 = cos (real)
                    S = Sp if ch else Sm
                    accE = ps.tile([128, 1024], F32, tag="acc")
                    accO = ps.tile([128, 1024], F32, tag="acc")
                    for i, a in enumerate(AORD):
                        wo = a * 256 + ch * 128
                        lE = W[0:64, wo:wo + 128]
                        lO = W[64:128, wo:wo + 128]
                        fE = (i == NA - 1) and ch == 0  # cos: edge MM stops
                        nc.tensor.matmul(accE[:, 0:C0], lE,
                                         S[0:64, a * TP:a * TP + C0],
                                         start=(i == 0), stop=fE)
                        nc.tensor.matmul(accO[:, 0:C0], lO,
                                         S[64:128, a * TP:a * TP + C0],
                                         start=(i == 0), stop=(i == NA - 1))
                        nc.tensor.matmul(accE[:, 512:512 + C1], lE,
                                         S[0:64, a * TP + C0:a * TP + T],
                                         start=(i == 0), stop=fE)
                        nc.tensor.matmul(accO[:, 512:512 + C1], lO,
                                         S[64:128, a * TP + C0:a * TP + T],
                                         start=(i == 0), stop=(i == NA - 1))
                    if ch == 1:  # n=1024 edge term (even n -> accE)
                        nc.tensor.matmul(accE[:, 0:C0], SG[0:64, :],
                                         XA[0:64, 2:2 + C0],
                                         start=False, stop=True)
                        nc.tensor.matmul(accE[:, 512:512 + C1], SG[0:64, :],
                                         XA[0:64, 2 + C0:2 + T],
                                         start=False, stop=True)
                    c0 = 0 if ch else NKT * T
                    c8 = c0 + 8 * T
                    for lo, hi, bk in ((0, C0, 0), (C0, T, 512)):
                        # tensor_tensor cannot take two PSUM operands: stage
                        # accO to SBUF on the scalar engine, combine on DVE
                        nc.scalar.copy(Od[:, lo:hi], accO[:, bk:bk + hi - lo])
                        nc.vector.tensor_add(stCN[:, c0 + lo:c0 + hi],
                                             accE[:, bk:bk + hi - lo],
                                             Od[:, lo:hi])
                        nc.vector.tensor_sub(stCN[:, c8 + lo:c8 + hi],
                                             accE[:, bk:bk + hi - lo],
                                             Od[:, lo:hi])
                for s in range(2):  # ship k-tile 8 now -- ready this early
                    dst = out_d.ap().rearrange(
                        "(p sk) t -> p sk t", sk=2 * NKT)[
                        :, s * NKT + 8:s * NKT + 9, :]
                    nc.sync.dma_start(
                        dst, stCN[:, (s * NKT + 8) * T:(s * NKT + 9) * T]
                        .rearrange("p (kt t) -> p kt t", kt=1))

                for kt in range(1, 8):
                    for ch in range(2):  # 0 = sin (imag), 1 = cos (real)
                        acc = ps.tile([128, 1024], F32, tag="acc")
                        S = Sp if ch else Sm
                        col = kt * T if ch == 1 else NKT * T + kt * T
                        last = kt == 7
                        if last and ch == 1:
                            # bank-split order: finish PSUM bank A first so its
                            # evacuation + DMA overlap bank B's matmuls -- only
                            # a 65KB DMA then trails the final matmul
                            for i, a in enumerate(AORD):
                                wo = kt * 2048 + a * 256 + ch * 128
                                nc.tensor.matmul(acc[:, 0:C0],
                                                 W[:, wo:wo + 128],
                                                 S[:, a * TP:a * TP + C0],
                                                 start=(i == 0),
                                                 stop=(i == NA - 1))
                            nc.vector.tensor_add(stCN[:, col:col + C0],
                                                 acc[:, 0:C0], Ev[:, 0:C0])
                            dst = out_d.ap().rearrange(
                                "(p sk) t -> p sk t", sk=2 * NKT)[:, kt:kt + 1, :]
                            nc.sync.dma_start(
                                dst[:, :, 0:C0],
                                stCN[:, col:col + C0].rearrange(
                                    "p (kt t) -> p kt t", kt=1))
                            for i, a in enumerate(AORD):
                                wo = kt * 2048 + a * 256 + ch * 128
                                nc.tensor.matmul(acc[:, 512:512 + C1],
                                                 W[:, wo:wo + 128],
                                                 S[:, a * TP + C0:a * TP + T],
                                                 start=(i == 0),
                                                 stop=(i == NA - 1))
                            nc.vector.tensor_add(stCN[:, col + C0:col + T],
                                                 acc[:, 512:512 + C1],
                                                 Ev[:, C0:T])
                            nc.sync.dma_start(
                                dst[:, :, C0:T],
                                stCN[:, col + C0:col + T].rearrange(
                                    "p (kt t) -> p kt t", kt=1))
                            continue
                        for i, a in enumerate(AORD):
                            wo = kt * 2048 + a * 256 + ch * 128
                            lhsT = W[:, wo:wo + 128]
                            nc.tensor.matmul(acc[:, 0:C0], lhsT,
                                             S[:, a * TP:a * TP + C0],
                                             start=(i == 0), stop=(i == NA - 1))
                            nc.tensor.matmul(acc[:, 512:512 + C1], lhsT,
                                             S[:, a * TP + C0:a * TP + T],
                                             start=(i == 0), stop=(i == NA - 1))
                        if ch == 1:
                            nc.vector.tensor_add(stCN[:, col:col + C0],
                                                 acc[:, 0:C0], Ev[:, 0:C0])
                            nc.vector.tensor_add(stCN[:, col + C0:col + T],
                                                 acc[:, 512:512 + C1],
                                                 Ev[:, C0:T])
                        else:
                            nc.scalar.copy(stCN[:, col:col + C0],
                                           acc[:, 0:C0])
                            nc.scalar.copy(stCN[:, col + C0:col + T],
                                           acc[:, 512:512 + C1])
                            if last:  # sin section ships during cos matmuls
                                dst = out_d.ap().rearrange(
                                    "(p sk) t -> p sk t", sk=2 * NKT)[
                                    :, NKT + kt:NKT + kt + 1, :]
                                nc.sync.dma_start(
                                    dst, stCN[:, col:col + T].rearrange(
                                        "p (kt t) -> p kt t", kt=1))
                    # stream finished k-tiles out (one DMA covers the cos and
                    # sin sections); waves shrink near the end so the last DMA
                    # tail after the final matmul is small
                    waves = {2: (0, 3), 5: (3, 3), 6: (6, 1)}
                    if kt in waves:
                        w0, nw = waves[kt]
                        for s in range(2):
                            cols = slice(s * NKT * T + w0 * T,
                                         s * NKT * T + (w0 + nw) * T)
                            dst = out_d.ap().rearrange(
                                "(p sk) t -> p sk t", sk=2 * NKT)[
                                :, s * NKT + w0:s * NKT + w0 + nw, :]
                            nc.sync.dma_start(
                                dst, stCN[:, cols].rearrange(
                                    "p (kt t) -> p kt t", kt=nw))
    nc.compile()
    return nc


def host_prep(x, wsin, wcos):
    """Marshal full inputs into per-core input maps (pure data movement)."""
    x = np.asarray(x, dtype=np.float32)
    B = x.shape[0]
    xp = np.pad(x, ((0, 0), (1024, 1024)), mode="reflect")
    # layout A: XA[p, j] = xp[128j + p]
    XA = np.ascontiguousarray(xp.reshape(B, 2064, 128).transpose(0, 2, 1))
    # reversed layout: XR[p, j] = xp[128(j+1) - p] (out-of-range -> 0, unused)
    j = np.arange(2068); p = np.arange(128)
    idx = 128 * (j[None, :] + 1) - p[:, None]
    oob = idx >= L_PAD
    idx = np.where(oob, 0, idx)
    XR = xp[:, idx]
    XR[:, oob] = 0.0

    def phase_split(M, pitch=PH):
        B_, P_, C = M.shape
        out = np.zeros((B_, P_, 4, pitch), dtype=np.float32)
        for ph in range(4):
            col = M[:, :, ph::4]
            out[:, :, ph, :col.shape[2]] = col
        return np.ascontiguousarray(
            out.reshape(B_, P_, 4 * pitch).astype(np.float16))

    # parity permutation: within every 128-sample block, even offsets to
    # partitions 0..63 and odd to 64..127 (kernel contracts halves separately)
    perm = np.concatenate([np.arange(0, 128, 2), np.arange(1, 128, 2)])
    XA4 = phase_split(XA[:, perm, :])
    XR4 = phase_split(XR[:, perm, :])
    WT = np.concatenate([np.asarray(wsin).T[:1024, :NKT * 128],
                         np.asarray(wcos).T[:1024, :NKT * 128]],
                        axis=1).astype(np.float16)          # (1024, 2304)
    WT = WT.reshape(NA, 128, 2 * NKT * 128)[:, perm, :].reshape(1024, -1)
    # block by k-tile so each 512KB block is one DMA and group kt only
    # depends on its own block: WH[kt*128+c', a*256+ch*128+p]; k-tile 8 is
    # derived on-chip from tile 0's parity halves, so only blocks 0..7 ship
    WH = WT.reshape(NA, 128, 2, NKT, 128).transpose(3, 1, 0, 2, 4)
    WT = np.ascontiguousarray(WH.reshape(NKT * 128, 2048)[:8 * 128])
    sgn = np.zeros((128, 128), dtype=np.float16)
    sgn[0, :] = (-1.0) ** np.arange(128)
    return [{"xa": XA4[b], "xr": XR4[b], "wt": WT, "sgn": sgn} for b in range(B)]


def assemble(results):
    """Gather per-core outputs into the full (real, -imag) pair."""
    B = len(results)
    R = np.empty((B, 2048, T), np.float32)
    I = np.empty((B, 2048, T), np.float32)
    for b in range(B):
        # p-major rows: row p*2*NKT + s*NKT + kt -> bin (s, kt*128 + p)
        o = results[b]["outRI"].astype(np.float32)
        o = o.reshape(128, 2, NKT, T).transpose(1, 2, 0, 3).reshape(2, 1152, T)
        oR, oI = o[0], o[1]              # oI holds +imag rows
        R[b, :1152] = oR
        R[b, 1152:] = oR[896:0:-1]       # cos mirror: copy
        I[b, :1152] = -oI
        I[b, 1152:] = oI[896:0:-1]       # sin mirror of -imag
    return R, I


class _Runner:
    """Build once, jit once, run many (shard_map over the 8 cores)."""

    def __init__(self, reps=1):
        import jax
        from jax.sharding import Mesh, PartitionSpec
        from jax.experimental.shard_map import shard_map
        from concourse.bass2jax import _bass_exec_p, install_neuronx_cc_hook

        install_neuronx_cc_hook()
        self.jax = jax
        nc = build_nc(reps=reps)
        self.nc = nc
        in_names, out_names, out_avals = [], [], []
        for alloc in nc.m.functions[0].allocations:
            if not isinstance(alloc, mybir.MemoryLocationSet):
                continue
            name = alloc.memorylocations[0].name
            if alloc.kind == "ExternalInput":
                in_names.append(name)
            elif alloc.kind == "ExternalOutput":
                out_names.append(name)
                out_avals.append(jax.core.ShapedArray(
                    tuple(alloc.tensor_shape), mybir.dt.np(alloc.dtype)))
        self.in_names, self.out_names, self.out_avals = in_names, out_names, out_avals
        n_params = len(in_names)
        all_names = in_names + out_names

        def _body(*args):
            outs = _bass_exec_p.bind(
                *args,
                out_avals=tuple(out_avals),
                in_names=tuple(all_names),
                out_names=tuple(out_names),
                lowering_input_output_aliases=(),
                sim_require_finite=True,
                sim_require_nnan=True,
                nc=nc,
            )
            return tuple(outs)

        devices = jax.devices()[:N_CORES]
        mesh = Mesh(np.asarray(devices), ("core",))
        n_outs = len(out_names)
        self._fn = jax.jit(
            shard_map(_body, mesh=mesh,
                      in_specs=(PartitionSpec("core"),) * (n_params + n_outs),
                      out_specs=(PartitionSpec("core"),) * n_outs,
                      check_rep=False),
            keep_unused=True,
        )
        self._zeros = [np.zeros((N_CORES * a.shape[0], *a.shape[1:]), a.dtype)
                       for a in out_avals]

    def prepare(self, in_maps):
        pid = self.nc.partition_id_tensor.name if self.nc.partition_id_tensor else None
        in_maps = [
            dict(m, **({pid: np.array([[c]], dtype=np.uint32)} if pid else {}))
            for c, m in enumerate(in_maps)
        ]
        concat = [np.concatenate([np.asarray(m[name]) for m in in_maps], axis=0)
                  for name in self.in_names]
        self._args = [self.jax.device_put(a) for a in concat + self._zeros]
        self.jax.block_until_ready(self._args)

    def run(self):
        out = self._fn(*self._args)
        self.jax.block_until_ready(out)
        return out

    def results(self, out):
        res = []
        for c in range(N_CORES):
            d = {}
            for i, name in enumerate(self.out_names):
                a = np.asarray(out[i])
                d[name] = a.reshape(N_CORES, *self.out_avals[i].shape)[c]
            res.append(d)
        return res


_RUNNER = None


def kernel(x, wsin, wcos):
    """Full inputs in, full output out: returns (real, -imag) as in reference."""
    global _RUNNER
    if _RUNNER is None:
        _RUNNER = _Runner(reps=1)
    ins = host_prep(x, wsin, wcos)
    _RUNNER.prepare(ins)
    out = _RUNNER.run()
    R, I = assemble(_RUNNER.results(out))
    return R, I


# revision 36
# speedup vs baseline: 1.1005x; 1.0417x over previous
"""Trainium2 Bass kernel for batched windowed DFT (STFT-as-GEMM).

Problem: for each batch row of x (8, 262144), reflect-pad by 1024, frame into
513 overlapping windows (len 2048, hop 512), and multiply by dense Hann-windowed
sin/cos DFT matrices (2048x2048):  real = wcos @ frames^T, out = (real, -imag).

Strategy (one batch per NeuronCore, 8 cores):
  * Hermitian symmetry of the real-input DFT: compute bins k=0..1151 only;
    rows 1152..2047 are mirrors (real: copy, imag: sign flip) of rows 896..1.
    The imag mirror is derived on host from the shipped rows (no extra DMA).
  * Even/odd fold of the contraction: the Hann-windowed basis obeys
    w[k, 2048-n] = +/- w[k, n], so contract only n=1..1023 against folded
    frames S+- = f[n] -+ f[2048-n] (computed on-chip by the vector engine).
    The n=1024 edge term is a rank-1 tile (+-1)^k * f[1024+512t] computed once
    on the PE and added during PSUM evacuation.
  * fp16 everywhere off-chip (signal in, weights, spectra out) -- halves DMA;
    fp32 PSUM accumulation on-chip keeps error ~5e-4.
  * 257/256 column split per PSUM bank pair (no 2-wide tail matmuls).
  * Outputs stream out in 3 kt-waves overlapped with later tiles' compute.
"""
import numpy as np

import concourse.bacc as bacc
import concourse.mybir as mybir
import concourse.tile as tile

F32 = mybir.dt.float32
F16 = mybir.dt.float16
T = 513          # frames
TP = 514         # padded column pitch for the folded-signal tiles
PH = 520         # per-phase column pitch of the phase-split signal layout
NKT = 9          # k tiles computed directly: k = 0..1151
NA = 8           # folded contraction chunks of 128 (n = 0..1023)
N_CORES = 8
L_PAD = 264192   # 262144 + 2*1024
C0, C1 = 257, 256  # frame split across the two PSUM banks of a group


def build_nc(reps=1):
    nc = bacc.Bacc("TRN2", target_bir_lowering=False, debug=False,
                   num_devices=N_CORES)
    xa_d = nc.dram_tensor("xa", [128, 4 * PH], F16, kind="ExternalInput")
    xr_d = nc.dram_tensor("xr", [128, 4 * PH], F16, kind="ExternalInput")
    wt_d = nc.dram_tensor("wt", [8 * 128, 2048], F16, kind="ExternalInput")
    sgn_d = nc.dram_tensor("sgn", [128, 128], F16, kind="ExternalInput")
    # p-major spectra: row p*2*NKT + s*NKT + kt holds bins (s, kt*128+p);
    # p-major keeps each output wave's DMA runs contiguous (nw*1026B per
    # partition instead of 1026B), host unpermutes for free.
    out_d = nc.dram_tensor("outRI", [128 * 2 * NKT, T], F16,
                           kind="ExternalOutput")

    with tile.TileContext(nc) as tc:
        with (
            tc.tile_pool(name="xin", bufs=1) as xin,
            tc.tile_pool(name="wts", bufs=1) as wts,
            tc.tile_pool(name="sbf", bufs=1) as sbf,
            tc.tile_pool(name="stag", bufs=1) as stag,
            tc.tile_pool(name="ps", bufs=3, space="PSUM") as ps,
            tc.tile_pool(name="pse", bufs=1, space="PSUM") as pse,
        ):
            XA = xin.tile([128, 4 * PH], F16, tag="XA")
            XR = xin.tile([128, 4 * PH], F16, tag="XR")
            SG = xin.tile([128, 128], F16, tag="SG")
            W = wts.tile([128, 8 * 2048], F16, tag="W")
            Sm = sbf.tile([128, NA * TP], F16, tag="Sm")
            Sp = sbf.tile([128, NA * TP], F16, tag="Sp")
            Ev = sbf.tile([128, T], F16, tag="Ev")
            Od = sbf.tile([128, T], F32, tag="Od")  # accO staging (kt0 split)
            stCN = stag.tile([128, 2 * NKT * T], F16, tag="stCN")

            for _rep in range(reps):
                # One HWDGE ring, explicit order: W block 0 first (first-group
                # matmuls need it), then signal halves in fold order (folds
                # a in {0,1,4,5} need only XA phases 0-1 + XR phases 2-3),
                # then the remaining weight blocks.
                nc.sync.dma_start(W[:, 0:2048], wt_d.ap()[0:128, :])
                nc.sync.dma_start(XA[:, 0:PH], xa_d.ap()[:, 0:PH])
                nc.sync.dma_start(XR[:, 3 * PH:4 * PH],
                                  xr_d.ap()[:, 3 * PH:4 * PH])
                nc.sync.dma_start(XA[:, PH:2 * PH], xa_d.ap()[:, PH:2 * PH])
                nc.sync.dma_start(XR[:, 2 * PH:3 * PH],
                                  xr_d.ap()[:, 2 * PH:3 * PH])
                nc.sync.dma_start(SG[:], sgn_d.ap())
                nc.sync.dma_start(XA[:, 2 * PH:4 * PH],
                                  xa_d.ap()[:, 2 * PH:4 * PH])
                nc.sync.dma_start(XR[:, 0:2 * PH], xr_d.ap()[:, 0:2 * PH])
                for kt in range(1, 8):
                    nc.sync.dma_start(W[:, kt * 2048:(kt + 1) * 2048],
                                      wt_d.ap()[kt * 128:(kt + 1) * 128, :])

                # folds: S-+[a][p, t] = f[128a+p] -+ f[2048-(128a+p)] per frame t
                AORD = [0, 4, 1, 5, 2, 3, 6, 7]  # DMA-arrival order
                for a in AORD:
                    xa_s = XA[:, (a % 4) * PH + a // 4:(a % 4) * PH + a // 4 + TP]
                    ph = (15 - a) % 4
                    off = (15 - a) // 4
                    xr_s = XR[:, ph * PH + off:ph * PH + off + TP]
                    nc.vector.tensor_sub(Sm[:, a * TP:a * TP + TP], xa_s, xr_s)
                    nc.vector.tensor_add(Sp[:, a * TP:a * TP + TP], xa_s, xr_s)

                # rank-1 edge tile Ev[p, t] = (-1)^p * xp[512t + 1024], built
                # once on the PE (SG row 0 = (-1)^p) and added to every cos
                # group's evacuation: n=1024 term, wcos[k,1024] = (-1)^k.
                eps = pse.tile([128, 1024], F32, tag="eps")
                nc.tensor.matmul(eps[:, 0:C0], SG[:], XA[:, 2:2 + C0],
                                 start=True, stop=True)
                nc.tensor.matmul(eps[:, 512:512 + C1], SG[:],
                                 XA[:, 2 + C0:2 + T], start=True, stop=True)
                nc.vector.tensor_copy(Ev[:, 0:C0], eps[:, 0:C0])
                nc.vector.tensor_copy(Ev[:, C0:T], eps[:, 512:512 + C1])

                # ---- k-tile 0 with even/odd parity split: the contraction
                # rows are host-permuted so partitions 0..63 carry even n and
                # 64..127 odd n; K=64 matmuls to PE row-groups {0,1} / {2,3}
                # run concurrently, and k-tile 8 = E - O comes out free
                # (w[k+1024, n] = (-1)^n w[k, n]).
                for ch in range(2):  # 0 = sin (imag), 1 = cos (real)
                    S = Sp if ch else Sm
                    accE = ps.tile([128, 1024], F32, tag="acc")
                    accO = ps.tile([128, 1024], F32, tag="acc")
                    for i, a in enumerate(AORD):
                        wo = a * 256 + ch * 128
                        lE = W[0:64, wo:wo + 128]
                        lO = W[64:128, wo:wo + 128]
                        fE = (i == NA - 1) and ch == 0  # cos: edge MM stops
                        nc.tensor.matmul(accE[:, 0:C0], lE,
                                         S[0:64, a * TP:a * TP + C0],
                                         start=(i == 0), stop=fE)
                        nc.tensor.matmul(accO[:, 0:C0], lO,
                                         S[64:128, a * TP:a * TP + C0],
                                         start=(i == 0), stop=(i == NA - 1))
                        nc.tensor.matmul(accE[:, 512:512 + C1], lE,
                                         S[0:64, a * TP + C0:a * TP + T],
                                         start=(i == 0), stop=fE)
                        nc.tensor.matmul(accO[:, 512:512 + C1], lO,
                                         S[64:128, a * TP + C0:a * TP + T],
                                         start=(i == 0), stop=(i == NA - 1))
                    if ch == 1:  # n=1024 edge term (even n -> accE)
                        nc.tensor.matmul(accE[:, 0:C0], SG[0:64, :],
                                         XA[0:64, 2:2 + C0],
                                         start=False, stop=True)
                        nc.tensor.matmul(accE[:, 512:512 + C1], SG[0:64, :],
                                         XA[0:64, 2 + C0:2 + T],
                                         start=False, stop=True)
                    c0 = 0 if ch else NKT * T
                    c8 = c0 + 8 * T
                    for lo, hi, bk in ((0, C0, 0), (C0, T, 512)):
                        # tensor_tensor cannot take two PSUM operands: stage
                        # accO to SBUF on the scalar engine, combine on DVE
                        nc.scalar.copy(Od[:, lo:hi], accO[:, bk:bk + hi - lo])
                        nc.vector.tensor_add(stCN[:, c0 + lo:c0 + hi],
                                             accE[:, bk:bk + hi - lo],
                                             Od[:, lo:hi])
                        nc.vector.tensor_sub(stCN[:, c8 + lo:c8 + hi],
                                             accE[:, bk:bk + hi - lo],
                                             Od[:, lo:hi])
                for s in range(2):  # ship k-tile 8 now -- ready this early
                    dst = out_d.ap().rearrange(
                        "(p sk) t -> p sk t", sk=2 * NKT)[
                        :, s * NKT + 8:s * NKT + 9, :]
                    nc.sync.dma_start(
                        dst, stCN[:, (s * NKT + 8) * T:(s * NKT + 9) * T]
                        .rearrange("p (kt t) -> p kt t", kt=1))

                for kt in range(1, 8):
                    for ch in range(2):  # 0 = sin (imag), 1 = cos (real)
                        acc = ps.tile([128, 1024], F32, tag="acc")
                        S = Sp if ch else Sm
                        col = kt * T if ch == 1 else NKT * T + kt * T
                        last = kt == 7
                        if last and ch == 1:
                            # bank-split order: finish PSUM bank A first so its
                            # evacuation + DMA overlap bank B's matmuls -- only
                            # a 65KB DMA then trails the final matmul
                            for i, a in enumerate(AORD):
                                wo = kt * 2048 + a * 256 + ch * 128
                                nc.tensor.matmul(acc[:, 0:C0],
                                                 W[:, wo:wo + 128],
                                                 S[:, a * TP:a * TP + C0],
                                                 start=(i == 0),
                                                 stop=(i == NA - 1))
                            nc.vector.tensor_add(stCN[:, col:col + C0],
                                                 acc[:, 0:C0], Ev[:, 0:C0])
                            dst = out_d.ap().rearrange(
                                "(p sk) t -> p sk t", sk=2 * NKT)[:, kt:kt + 1, :]
                            nc.sync.dma_start(
                                dst[:, :, 0:C0],
                                stCN[:, col:col + C0].rearrange(
                                    "p (kt t) -> p kt t", kt=1))
                            for i, a in enumerate(AORD):
                                wo = kt * 2048 + a * 256 + ch * 128
                                nc.tensor.matmul(acc[:, 512:512 + C1],
                                                 W[:, wo:wo + 128],
                                                 S[:, a * TP + C0:a * TP + T],
                                                 start=(i == 0),
                                                 stop=(i == NA - 1))
                            nc.vector.tensor_add(stCN[:, col + C0:col + T],
                                                 acc[:, 512:512 + C1],
                                                 Ev[:, C0:T])
                            nc.sync.dma_start(
                                dst[:, :, C0:T],
                                stCN[:, col + C0:col + T].rearrange(
                                    "p (kt t) -> p kt t", kt=1))
                            continue
                        for i, a in enumerate(AORD):
                            wo = kt * 2048 + a * 256 + ch * 128
                            lhsT = W[:, wo:wo + 128]
                            nc.tensor.matmul(acc[:, 0:C0], lhsT,
                                             S[:, a * TP:a * TP + C0],
                                             start=(i == 0), stop=(i == NA - 1))
                            nc.tensor.matmul(acc[:, 512:512 + C1], lhsT,
                                             S[:, a * TP + C0:a * TP + T],
                                             start=(i == 0), stop=(i == NA - 1))
                        if ch == 1:
                            nc.vector.tensor_add(stCN[:, col:col + C0],
                                                 acc[:, 0:C0], Ev[:, 0:C0])
                            nc.vector.tensor_add(stCN[:, col + C0:col + T],
                                                 acc[:, 512:512 + C1],
                                                 Ev[:, C0:T])
                        else:
                            nc.scalar.copy(stCN[:, col:col + C0],
                                           acc[:, 0:C0])
                            nc.scalar.copy(stCN[:, col + C0:col + T],
                                           acc[:, 512:512 + C1])
                            if last:  # sin section ships during cos matmuls
                                dst = out_d.ap().rearrange(
                                    "(p sk) t -> p sk t", sk=2 * NKT)[
                                    :, NKT + kt:NKT + kt + 1, :]
                                nc.sync.dma_start(
                                    dst, stCN[:, col:col + T].rearrange(
                                        "p (kt t) -> p kt t", kt=1))
                    # stream finished k-tiles out (one DMA covers the cos and
                    # sin sections); waves shrink near the end so the last DMA
                    # tail after the final matmul is small
                    waves = {2: (0, 3), 5: (3, 3), 6: (6, 1)}
                    if kt in waves:
                        w0, nw = waves[kt]
                        for s in range(2):
                            cols = slice(s * NKT * T + w0 * T,
                                         s * NKT * T + (w0 + nw) * T)
                            dst = out_d.ap().rearrange(
                                "(p sk) t -> p sk t", sk=2 * NKT)[
                                :, s * NKT + w0:s * NKT + w0 + nw, :]
                            nc.sync.dma_start(
                                dst, stCN[:, cols].rearrange(
                                    "p (kt t) -> p kt t", kt=nw))
    nc.compile()
    return nc


def host_prep(x, wsin, wcos):
    """Marshal full inputs into per-core input maps (pure data movement)."""
    x = np.asarray(x, dtype=np.float32)
    B = x.shape[0]
    xp = np.pad(x, ((0, 0), (1024, 1024)), mode="reflect")
    # layout A: XA[p, j] = xp[128j + p]
    XA = np.ascontiguousarray(xp.reshape(B, 2064, 128).transpose(0, 2, 1))
    # reversed layout: XR[p, j] = xp[128(j+1) - p] (out-of-range -> 0, unused)
    j = np.arange(2068); p = np.arange(128)
    idx = 128 * (j[None, :] + 1) - p[:, None]
    oob = idx >= L_PAD
    idx = np.where(oob, 0, idx)
    XR = xp[:, idx]
    XR[:, oob] = 0.0

    def phase_split(M, pitch=PH):
        B_, P_, C = M.shape
        out = np.zeros((B_, P_, 4, pitch), dtype=np.float32)
        for ph in range(4):
            col = M[:, :, ph::4]
            out[:, :, ph, :col.shape[2]] = col
        return np.ascontiguousarray(
            out.reshape(B_, P_, 4 * pitch).astype(np.float16))

    # parity permutation: within every 128-sample block, even offsets to
    # partitions 0..63 and odd to 64..127 (kernel contracts halves separately)
    perm = np.concatenate([np.arange(0, 128, 2), np.arange(1, 128, 2)])
    XA4 = phase_split(XA[:, perm, :])
    XR4 = phase_split(XR[:, perm, :])
    WT = np.concatenate([np.asarray(wsin).T[:1024, :NKT * 128],
                         np.asarray(wcos).T[:1024, :NKT * 128]],
                        axis=1).astype(np.float16)          # (1024, 2304)
    WT = WT.reshape(NA, 128, 2 * NKT * 128)[:, perm, :].reshape(1024, -1)
    # block by k-tile so each 512KB block is one DMA and group kt only
    # depends on its own block: WH[kt*128+c', a*256+ch*128+p]; k-tile 8 is
    # derived on-chip from tile 0's parity halves, so only blocks 0..7 ship
    WH = WT.reshape(NA, 128, 2, NKT, 128).transpose(3, 1, 0, 2, 4)
    WT = np.ascontiguousarray(WH.reshape(NKT * 128, 2048)[:8 * 128])
    sgn = np.zeros((128, 128), dtype=np.float16)
    sgn[0, :] = (-1.0) ** np.arange(128)
    return [{"xa": XA4[b], "xr": XR4[b], "wt": WT, "sgn": sgn} for b in range(B)]


def assemble(results):
    """Gather per-core outputs into the full (real, -imag) pair."""
    B = len(results)
    R = np.empty((B, 2048, T), np.float32)
    I = np.empty((B, 2048, T), np.float32)
    for b in range(B):
        # p-major rows: row p*2*NKT + s*NKT + kt -> bin (s, kt*128 + p)
        o = results[b]["outRI"].astype(np.float32)
        o = o.reshape(128, 2, NKT, T).transpose(1, 2, 0, 3).reshape(2, 1152, T)
        oR, oI = o[0], o[1]              # oI holds +imag rows
        R[b, :1152] = oR
        R[b, 1152:] = oR[896:0:-1]       # cos mirror: copy
        I[b, :1152] = -oI
        I[b, 1152:] = oI[896:0:-1]       # sin mirror of -imag
    return R, I


class _Runner:
    """Build once, jit once, run many (shard_map over the 8 cores)."""

    def __init__(self, reps=1):
        import jax
        from jax.sharding import Mesh, PartitionSpec
        from jax.experimental.shard_map import shard_map
        from concourse.bass2jax import _bass_exec_p, install_neuronx_cc_hook

        install_neuronx_cc_hook()
        self.jax = jax
        nc = build_nc(reps=reps)
        self.nc = nc
        in_names, out_names, out_avals = [], [], []
        for alloc in nc.m.functions[0].allocations:
            if not isinstance(alloc, mybir.MemoryLocationSet):
                continue
            name = alloc.memorylocations[0].name
            if alloc.kind == "ExternalInput":
                in_names.append(name)
            elif alloc.kind == "ExternalOutput":
                out_names.append(name)
                out_avals.append(jax.core.ShapedArray(
                    tuple(alloc.tensor_shape), mybir.dt.np(alloc.dtype)))
        self.in_names, self.out_names, self.out_avals = in_names, out_names, out_avals
        n_params = len(in_names)
        all_names = in_names + out_names

        def _body(*args):
            outs = _bass_exec_p.bind(
                *args,
                out_avals=tuple(out_avals),
                in_names=tuple(all_names),
                out_names=tuple(out_names),
                lowering_input_output_aliases=(),
                sim_require_finite=True,
                sim_require_nnan=True,
                nc=nc,
            )
            return tuple(outs)

        devices = jax.devices()[:N_CORES]
        mesh = Mesh(np.asarray(devices), ("core",))
        n_outs = len(out_names)
        self._fn = jax.jit(
            shard_map(_body, mesh=mesh,
                      in_specs=(PartitionSpec("core"),) * (n_params + n_outs),
                      out_specs=(PartitionSpec("core"),) * n_outs,
                      check_rep=False),
            keep_unused=True,
        )
        self._zeros = [np.zeros((N_CORES * a.shape[0], *a.shape[1:]), a.dtype)
                       for a in out_avals]

    def prepare(self, in_maps):
        pid = self.nc.partition_id_tensor.name if self.nc.partition_id_tensor else None
        in_maps = [
            dict(m, **({pid: np.array([[c]], dtype=np.uint32)} if pid else {}))
            for c, m in enumerate(in_maps)
        ]
        concat = [np.concatenate([np.asarray(m[name]) for m in in_maps], axis=0)
                  for name in self.in_names]
        self._args = [self.jax.device_put(a) for a in concat + self._zeros]
        self.jax.block_until_ready(self._args)

    def run(self):
        out = self._fn(*self._args)
        self.jax.block_until_ready(out)
        return out

    def results(self, out):
        res = []
        for c in range(N_CORES):
            d = {}
            for i, name in enumerate(self.out_names):
                a = np.asarray(out[i])
                d[name] = a.reshape(N_CORES, *self.out_avals[i].shape)[c]
            res.append(d)
        return res


_RUNNER = None


def kernel(x, wsin, wcos):
    """Full inputs in, full output out: returns (real, -imag) as in reference."""
    global _RUNNER
    if _RUNNER is None:
        _RUNNER = _Runner(reps=1)
    ins = host_prep(x, wsin, wcos)
    _RUNNER.prepare(ins)
    out = _RUNNER.run()
    R, I = assemble(_RUNNER.results(out))
    return R, I
